# revision 2
# baseline (speedup 1.0000x reference)
"""DAM-Softmax loss kernel for Trainium2 (Bass/Tile), 8-core data parallel.

Math (per sample b, target t = label[b]):
    cos_t  = costh[b, t]
    delta  = (MARGIN/LAMDA) * exp(1 - cos_t)
    logits = S * costh, with logits[b, t] = S * (cos_t - delta)
    loss_b = logsumexp(logits[b, :]) - S * (cos_t - delta)
    loss   = mean_b loss_b

costh is bounded in [0, 1), so M = 1.0 is the stability shift:
    ssum   = sum_j exp(S*(costh[b,j] - M))
    Z      = ssum - exp(S*(cos_t - M)) + exp(S*(cos_t - delta - M))
    loss_b = S*M + ln(Z) - S*(cos_t - delta)

Performance structure (per core: [1024, 10000] shard, 10.24M exps):

* The bulk ssum term only needs ~1% accuracy (harness tolerance is 2e-2
  on the final scalar; per-element errors average out over 10000
  columns), so the stream tensor is staged in HBM as fp8 e4m3 -- 4x
  fewer bytes than f32.  The fp8 DMA stream (10.24 MB/core) runs at the
  aggregate 16-engine DMA bus limit, ~360-370 GB/s/core = ~27.5 us --
  this is the roofline the kernel targets.
* The 10.24M elementwise exps are split across THREE engines by column
  range so the total elementwise capacity (~2.9 elem/ns) exceeds the
  DMA rate:
    - ACT (ScalarE): true exp via activation table, fused row-accum,
      ~0.78 ns/elem, on WA columns.
    - DVE (VectorE): Schraudolph exp2 pass1 on WD columns:
          i16 = rint(x * (S*log2e*128) + (127 - S*log2e)*128)
      (one tensor_scalar, f32 internal, int16 round-on-write, 1 elem/
      cycle), then pass2 for BOTH the DVE and Pool code ranges: bitcast
      i16 -> bf16 gives 2^z with mantissa-linear interpolation error;
      a second tensor_scalar row-accumulates it in 4x perf mode.
    - Pool (GPSIMD): the same Schraudolph pass1 on WP columns
      (bit-exact with DVE's, verified on HW) at ~0.72-0.83 ns/elem.
  The deterministic quantization/interpolation biases of the two paths
  (true-exp-of-fp8 vs Schraudolph-of-fp8) are removed by constant
  factors computed offline from the rounding rules (equidistribution
  within bins -- input-independent).
* Per-sample target terms (cos_t gather, margin, e1/e2, -S*ct_adj) are
  computed from the untouched f32 input: exact where it matters.

WA/WD/WP = 4200/1400/4400 balances ACT (WA cycles @ 1.2 GHz), DVE
(WD @ 1x + (WD+WP)/4 @ 4x, @ 0.96 GHz) and Pool (WP @ ~1.2 GHz): all
~26-27 us, just under the ~27.5 us DMA stream.

Sharding: batch dim split evenly across 8 NeuronCores (data parallel);
host mean-reduces the 8 x [128, 8] per-sample loss outputs.
"""

import numpy as np
import ml_dtypes

NCORES = 8
B, C = 8192, 10000
R = B // NCORES          # rows per core
P = 128                  # SBUF partitions
T = R // P               # row tiles per core
S = 15.0
MARGIN = 0.3
LAMDA = 2.0
DCOEF = MARGIN / LAMDA
MAXC = 1.0               # upper bound of costh (uniform [0,1)) used as exp shift

FP8 = ml_dtypes.float8_e4m3
LOG2E = float(np.log2(np.e))
SCHRA_A = S * LOG2E * 128.0             # pass1: code = rint(x*A + B)
SCHRA_B = (127.0 - S * LOG2E) * 128.0
WA = 4200                # columns handled by ACT (true exp)
WD = 1400                # columns handled by DVE (Schraudolph pass1)
WP = C - WA - WD         # columns handled by Pool (Schraudolph pass1)


def _schraudolph_np(x64):
    """Bit-exact numpy model of the DVE/Pool pass1 + pass2 pipeline."""
    codes = np.rint(x64 * SCHRA_A + SCHRA_B).astype(np.int16)
    return codes.view(ml_dtypes.bfloat16).astype(np.float64)


def _debias():
    """Constant bias factors of the two approximate paths vs true exp,
    for equidistributed in-bin rounding errors (input-independent).

    ACT path: fp8 quantization only.  DVE/Pool path: fp8 quantization +
    Schraudolph mantissa-linear interpolation + int16 rounding.
    """
    x = (np.arange(1 << 20, dtype=np.float64) + 0.5) / (1 << 20)
    xq = x.astype(np.float32).astype(FP8).astype(np.float64)
    num = np.exp(S * (x - 1.0)).sum()
    act = num / np.exp(S * (xq - 1.0)).sum()
    dve = num / _schraudolph_np(xq).sum()
    return float(act), float(dve)


DEBIAS_ACT, DEBIAS_DVE = _debias()

_NC_CACHE = {}


def _build_nc(repeat=1, big_bufs=4, loop_reps=1, wa=WA, wd=WD):
    # repeat > 1 re-streams the shard `repeat` times inside one NEFF; used by
    # the timing harness to infer per-pass device time from the wall-clock
    # slope (axon dispatch overhead cancels in the difference).  loop_reps > 1
    # additionally wraps the passes in a hardware For_i loop (amplifies
    # device time without growing the NEFF, at ~10% loop-sync tax).
    import concourse.bacc as bacc
    import concourse.bass as bass
    import concourse.mybir as mybir
    import concourse.tile as tile

    f32 = mybir.dt.float32
    fp8 = mybir.dt.float8e4
    bf16 = mybir.dt.bfloat16
    i16 = mybir.dt.int16
    i32 = mybir.dt.int32
    Act = mybir.ActivationFunctionType
    Alu = mybir.AluOpType

    wp = C - wa - wd
    wdp = wd + wp
    nc = bacc.Bacc(None, target_bir_lowering=False, debug=False)

    costh = nc.dram_tensor("costh", [R, C], f32, kind="ExternalInput")
    costh8 = nc.dram_tensor("costh8", [R, C], fp8, kind="ExternalInput")
    label = nc.dram_tensor("label", [R], i32, kind="ExternalInput")
    out = nc.dram_tensor("out", [P, 2 * T], f32, kind="ExternalOutput")

    with tile.TileContext(nc) as tc:
        with (
            tc.tile_pool(name="big", bufs=big_bufs) as big,
            tc.tile_pool(name="codes", bufs=2) as cpool,
            tc.tile_pool(name="small", bufs=1) as small,
        ):
            # bias vector for exp(S*x - S*M) activations
            neg_sm = small.tile([P, 1], f32)
            nc.vector.memset(neg_sm[:], -S * MAXC)

            # --- prologue: gather target cosines cos_t[p, t] = costh[t*P+p, label] ---
            label_sb = small.tile([P, T], i32)
            nc.gpsimd.dma_start(
                out=label_sb[:], in_=label[:].rearrange("(t p) -> p t", p=P)
            )
            # idx[p, t] = (t*P + p) * C + label  (flat element index), computed
            # in f32 (exact: values < 2^24) since iota steps are limited to i16.
            row_i = small.tile([P, T], i32)
            nc.gpsimd.iota(row_i[:], pattern=[[P, T]], base=0, channel_multiplier=1)
            row_f = small.tile([P, T], f32)
            nc.vector.tensor_copy(out=row_f[:], in_=row_i[:])
            lab_f = small.tile([P, T], f32)
            nc.vector.tensor_copy(out=lab_f[:], in_=label_sb[:])
            idx_f = small.tile([P, T], f32)
            nc.vector.scalar_tensor_tensor(
                out=idx_f[:], in0=row_f[:], scalar=float(C), in1=lab_f[:],
                op0=Alu.mult, op1=Alu.add,
            )
            idx = small.tile([P, T], i32)
            nc.vector.tensor_copy(out=idx[:], in_=idx_f[:])
            # one indirect DMA per column: HW honors only one index per
            # partition per gather (multi-column offset APs misbehave on HW)
            cos_t = small.tile([P, T], f32)
            for t in range(T):
                nc.gpsimd.indirect_dma_start(
                    out=cos_t[:, t:t + 1],
                    out_offset=None,
                    in_=costh[:, :],
                    in_offset=bass.IndirectOffsetOnAxis(ap=idx[:, t:t + 1], axis=1),
                )

            # --- main loop: stream fp8 shard; per tile, one DMA of all C
            # columns, then ACT does true exp+accum on cols [0, wa), DVE
            # does Schraudolph pass1 on [wa, wa+wd), Pool on [wa+wd, C);
            # DVE pass2 accumulates both code ranges at 4x ---
            exp_scr = small.tile([P, wa], f32)     # ACT main out (scratch)
            scr16 = small.tile([P, wdp], bf16)     # DVE pass2 main out (scratch)
            sA = small.tile([P, T], f32)
            sD = small.tile([P, T], f32)

            def one_pass():
                for t in range(T):
                    x = big.tile([P, C], fp8, tag="x")
                    nc.sync.dma_start(
                        out=x[:], in_=costh8[t * P:(t + 1) * P, :]
                    )
                    codes = cpool.tile([P, wdp], i16, tag="codes")
                    nc.gpsimd.tensor_scalar(
                        out=codes[:, wd:wdp], in0=x[:, wa + wd:C],
                        scalar1=SCHRA_A, scalar2=SCHRA_B,
                        op0=Alu.mult, op1=Alu.add,
                    )
                    nc.scalar.activation(
                        out=exp_scr[:], in_=x[:, 0:wa], func=Act.Exp,
                        bias=neg_sm[:], scale=S,
                        accum_out=sA[:, t:t + 1],
                    )
                    nc.vector.tensor_scalar(
                        out=codes[:, 0:wd], in0=x[:, wa:wa + wd],
                        scalar1=SCHRA_A, scalar2=SCHRA_B,
                        op0=Alu.mult, op1=Alu.add,
                    )
                    nc.vector.tensor_scalar(
                        out=scr16[:], in0=codes[:].bitcast(bf16),
                        scalar1=1.0, scalar2=0.0, op0=Alu.mult, op1=Alu.add,
                        accum_out=sD[:, t:t + 1],
                    )

            if loop_reps > 1:
                with tc.For_i(0, loop_reps, 1):
                    for _rep in range(repeat):
                        one_pass()
            for _rep in range(repeat):
                one_pass()

            # --- tail ---
            # target-term math first (needs only cos_t, which the gather
            # produced while the stream ran; keeping these small ACTIVATEs
            # out of the prologue lets ACT start streaming immediately
            # instead of stalling on the gather chain):
            #   delta_e = exp(1 - cos_t);  ct_adj = cos_t - DCOEF * delta_e
            #   e12 = exp(S*(cos_t - M)) - exp(S*(ct_adj - M))
            delta_e = small.tile([P, T], f32)
            nc.scalar.activation(
                out=delta_e[:], in_=cos_t[:], func=Act.Exp, bias=1.0, scale=-1.0
            )
            ct_adj = small.tile([P, T], f32)
            nc.vector.scalar_tensor_tensor(
                out=ct_adj[:], in0=delta_e[:], scalar=-DCOEF, in1=cos_t[:],
                op0=Alu.mult, op1=Alu.add,
            )
            e1 = small.tile([P, T], f32)
            nc.scalar.activation(
                out=e1[:], in_=cos_t[:], func=Act.Exp, bias=neg_sm[:], scale=S
            )
            e2 = small.tile([P, T], f32)
            nc.scalar.activation(
                out=e2[:], in_=ct_adj[:], func=Act.Exp, bias=neg_sm[:], scale=S
            )
            e12 = small.tile([P, T], f32)
            nc.vector.tensor_sub(out=e12[:], in0=e1[:], in1=e2[:])

            # z = DEBIAS_ACT*sA + DEBIAS_DVE*sD - e12.  The final
            # ln(z) - S*ct_adj is finished on the HOST (O(B) work, where the
            # mean-reduction already happens): keeping Ln off the device
            # avoids a second ACT table load (~2.7 us) in the tail, since
            # the stream's Exp set does not contain Ln.
            zd = small.tile([P, T], f32)
            nc.vector.scalar_tensor_tensor(
                out=zd[:], in0=sD[:], scalar=DEBIAS_DVE, in1=e12[:],
                op0=Alu.mult, op1=Alu.subtract,
            )
            z = small.tile([P, T], f32)
            nc.vector.scalar_tensor_tensor(
                out=z[:], in0=sA[:], scalar=DEBIAS_ACT, in1=zd[:],
                op0=Alu.mult, op1=Alu.add,
            )
            nc.sync.dma_start(out=out[:, 0:T], in_=z[:])
            nc.sync.dma_start(out=out[:, T:2 * T], in_=ct_adj[:])

    nc.compile()
    return nc


def _get_nc():
    if "nc" not in _NC_CACHE:
        _NC_CACHE["nc"] = _build_nc()
    return _NC_CACHE["nc"]


def _full_inputs(costh, label):
    """Full (unsharded) input arrays keyed by dram tensor name."""
    costh = np.ascontiguousarray(costh, dtype=np.float32)
    return {
        "costh": costh,
        "costh8": costh.astype(FP8),
        "label": np.ascontiguousarray(label).astype(np.int32),
    }


def _run(costh_np, label_np, trace=False, **spmd_kwargs):
    from concourse.bass_utils import run_bass_kernel_spmd

    nc = _get_nc()
    full = _full_inputs(costh_np, label_np)
    in_maps = [
        {name: arr[k * R:(k + 1) * R] for name, arr in full.items()}
        for k in range(NCORES)
    ]
    # The first execution of a fresh NEFF through the axon tunnel
    # occasionally faults with NRT_EXEC_UNIT_UNRECOVERABLE; failures are
    # loud (exception, never silent corruption), so a bounded retry is safe.
    # A non-finite total also triggers a retry as extra insurance.
    last_exc = None
    for _attempt in range(3):
        try:
            res = run_bass_kernel_spmd(
                nc, in_maps, core_ids=list(range(NCORES)), trace=trace,
                **spmd_kwargs
            )
            total = 0.0
            for r in res.results:
                o = r["out"].astype(np.float64)
                zv, ctv = o[:, :T], o[:, T:]
                total += (np.log(zv) - S * ctv).sum()
            if np.isfinite(total):
                break
            last_exc = RuntimeError("non-finite loss from device")
        except Exception as exc:  # noqa: BLE001
            last_exc = exc
    else:
        raise last_exc
    loss = np.float32(total / B + S * MAXC)
    return loss, res


def kernel(costh, label):
    loss, _ = _run(costh, label)
    return loss


# revision 5
# speedup vs baseline: 1.3943x; 1.3943x over previous
"""DAM-Softmax loss kernel for Trainium2 (Bass/Tile), 8-core data parallel.

Math (per sample b, target t = label[b]):
    cos_t  = costh[b, t]
    delta  = (MARGIN/LAMDA) * exp(1 - cos_t)
    logits = S * costh, with logits[b, t] = S * (cos_t - delta)
    loss_b = logsumexp(logits[b, :]) - S * (cos_t - delta)
    loss   = mean_b loss_b

costh is bounded in [0, 1), so M = 1.0 is the stability shift:
    ssum   = sum_j exp(S*(costh[b,j] - M))
    Z      = ssum - exp(S*(cos_t - M)) + exp(S*(cos_t - delta - M))
    loss_b = S*M + ln(Z) - S*(cos_t - delta)

Performance structure (per core: [1024, 10000] shard, 10.24M exps):

* The bulk ssum term only needs ~1% accuracy (harness tolerance is 2e-2
  on the final scalar; per-element errors average out over 10000
  columns), so the stream tensor is staged in HBM as fp8 e4m3 -- 4x
  fewer bytes than f32.  The fp8 DMA stream (10.24 MB/core) runs at the
  aggregate 16-engine DMA bus limit, ~360-370 GB/s/core = ~27.5 us --
  this is the roofline the kernel targets.
* The 10.24M elementwise exps are split across THREE engines by column
  range so the total elementwise capacity (~2.9 elem/ns) exceeds the
  DMA rate:
    - ACT (ScalarE): true exp via activation table, fused row-accum,
      ~0.78 ns/elem, on WA columns.
    - DVE (VectorE): Schraudolph exp2 pass1 on WD columns:
          i16 = rint(x * (S*log2e*128) + (127 - S*log2e)*128)
      (one tensor_scalar, f32 internal, int16 round-on-write, 1 elem/
      cycle), then pass2 for BOTH the DVE and Pool code ranges: bitcast
      i16 -> bf16 gives 2^z with mantissa-linear interpolation error;
      a second tensor_scalar row-accumulates it in 4x perf mode.
    - Pool (GPSIMD): the same Schraudolph pass1 on WP columns
      (bit-exact with DVE's, verified on HW) at ~0.72-0.83 ns/elem.
  The deterministic quantization/interpolation biases of the two paths
  (true-exp-of-fp8 vs Schraudolph-of-fp8) are removed by constant
  factors computed offline from the rounding rules (equidistribution
  within bins -- input-independent).
* Per-sample target terms (cos_t gather, margin, e1/e2, -S*ct_adj) are
  computed from the untouched f32 input: exact where it matters.

WA/WD/WP = 4200/1400/4400 balances ACT (WA cycles @ 1.2 GHz), DVE
(WD @ 1x + (WD+WP)/4 @ 4x, @ 0.96 GHz) and Pool (WP @ ~1.2 GHz): all
~26-27 us, just under the ~27.5 us DMA stream.

Sharding: batch dim split evenly across 8 NeuronCores (data parallel);
host mean-reduces the 8 x [128, 8] per-sample loss outputs.
"""

import numpy as np
import ml_dtypes

NCORES = 8
B, C = 8192, 10000
R = B // NCORES          # rows per core
P = 128                  # SBUF partitions
T = R // P               # row tiles per core
S = 15.0
MARGIN = 0.3
LAMDA = 2.0
DCOEF = MARGIN / LAMDA
MAXC = 1.0               # upper bound of costh (uniform [0,1)) used as exp shift

FP8 = ml_dtypes.float8_e4m3
LOG2E = float(np.log2(np.e))
SCHRA_A = S * LOG2E * 128.0             # pass1: code = rint(x*A + B)
SCHRA_B = (127.0 - S * LOG2E) * 128.0
WA = 4580                # columns handled by ACT (true exp)
WD = 1420                # columns handled by DVE (Schraudolph pass1)
WP = C - WA - WD         # columns handled by Pool (Schraudolph pass1)


def _schraudolph_np(x64):
    """Bit-exact numpy model of the DVE/Pool pass1 + pass2 pipeline."""
    codes = np.rint(x64 * SCHRA_A + SCHRA_B).astype(np.int16)
    return codes.view(ml_dtypes.bfloat16).astype(np.float64)


def _debias():
    """Constant bias factors of the two approximate paths vs true exp,
    for equidistributed in-bin rounding errors (input-independent).

    ACT path: fp8 quantization only.  DVE/Pool path: fp8 quantization +
    Schraudolph mantissa-linear interpolation + int16 rounding.
    """
    x = (np.arange(1 << 20, dtype=np.float64) + 0.5) / (1 << 20)
    xq = x.astype(np.float32).astype(FP8).astype(np.float64)
    num = np.exp(S * (x - 1.0)).sum()
    act = num / np.exp(S * (xq - 1.0)).sum()
    dve = num / _schraudolph_np(xq).sum()
    return float(act), float(dve)


DEBIAS_ACT, DEBIAS_DVE = _debias()

_NC_CACHE = {}


def _build_nc(repeat=1, big_bufs=4, loop_reps=1, wa=WA, wd=WD):
    # repeat > 1 re-streams the shard `repeat` times inside one NEFF; used by
    # the timing harness to infer per-pass device time from the wall-clock
    # slope (axon dispatch overhead cancels in the difference).  loop_reps > 1
    # additionally wraps the passes in a hardware For_i loop (amplifies
    # device time without growing the NEFF, at ~10% loop-sync tax).
    import concourse.bacc as bacc
    import concourse.bass as bass
    import concourse.mybir as mybir
    import concourse.tile as tile

    f32 = mybir.dt.float32
    fp8 = mybir.dt.float8e4
    bf16 = mybir.dt.bfloat16
    i16 = mybir.dt.int16
    i32 = mybir.dt.int32
    Act = mybir.ActivationFunctionType
    Alu = mybir.AluOpType

    wp = C - wa - wd
    wdp = wd + wp
    nc = bacc.Bacc(None, target_bir_lowering=False, debug=False)

    costh = nc.dram_tensor("costh", [R, C], f32, kind="ExternalInput")
    costh8 = nc.dram_tensor("costh8", [R, C], fp8, kind="ExternalInput")
    label = nc.dram_tensor("label", [R], i32, kind="ExternalInput")
    out = nc.dram_tensor("out", [P, 2 * T], f32, kind="ExternalOutput")

    with tile.TileContext(nc) as tc:
        with (
            tc.tile_pool(name="big", bufs=big_bufs) as big,
            tc.tile_pool(name="codes", bufs=3) as cpool,
            tc.tile_pool(name="small", bufs=1) as small,
        ):
            # bias vector for exp(S*x - S*M) activations
            neg_sm = small.tile([P, 1], f32)
            nc.vector.memset(neg_sm[:], -S * MAXC)

            # --- prologue: gather target cosines cos_t[p, t] = costh[t*P+p, label] ---
            label_sb = small.tile([P, T], i32)
            nc.gpsimd.dma_start(
                out=label_sb[:], in_=label[:].rearrange("(t p) -> p t", p=P)
            )
            # idx[p, t] = (t*P + p) * C + label  (flat element index), computed
            # in f32 (exact: values < 2^24) since iota steps are limited to i16.
            row_i = small.tile([P, T], i32)
            nc.gpsimd.iota(row_i[:], pattern=[[P, T]], base=0, channel_multiplier=1)
            row_f = small.tile([P, T], f32)
            nc.vector.tensor_copy(out=row_f[:], in_=row_i[:])
            lab_f = small.tile([P, T], f32)
            nc.vector.tensor_copy(out=lab_f[:], in_=label_sb[:])
            idx_f = small.tile([P, T], f32)
            nc.vector.scalar_tensor_tensor(
                out=idx_f[:], in0=row_f[:], scalar=float(C), in1=lab_f[:],
                op0=Alu.mult, op1=Alu.add,
            )
            idx = small.tile([P, T], i32)
            nc.vector.tensor_copy(out=idx[:], in_=idx_f[:])
            # one indirect DMA per column: HW honors only one index per
            # partition per gather (multi-column offset APs misbehave on HW)
            cos_t = small.tile([P, T], f32)
            for t in range(T):
                nc.gpsimd.indirect_dma_start(
                    out=cos_t[:, t:t + 1],
                    out_offset=None,
                    in_=costh[:, :],
                    in_offset=bass.IndirectOffsetOnAxis(ap=idx[:, t:t + 1], axis=1),
                )

            # --- main loop: stream fp8 shard; per tile, one DMA of all C
            # columns, then ACT does true exp+accum on cols [0, wa), DVE
            # does Schraudolph pass1 on [wa, wa+wd), Pool on [wa+wd, C);
            # DVE pass2 accumulates both code ranges at 4x ---
            exp_scr = small.tile([P, wa], f32)     # ACT main out (scratch)
            scr16 = small.tile([P, wdp], bf16)     # DVE pass2 main out (scratch)
            sA = small.tile([P, T], f32)
            sD = small.tile([P, T], f32)

            def one_pass():
                # Software-pipelined: pass2 for tile t runs one tile behind
                # the pass1s, so the DVE queue never head-of-line blocks on
                # Pool finishing the current tile's codes.
                codes_by_t = {}

                def pass2(t):
                    nc.vector.tensor_scalar(
                        out=scr16[:], in0=codes_by_t.pop(t)[:].bitcast(bf16),
                        scalar1=1.0, scalar2=0.0, op0=Alu.mult, op1=Alu.add,
                        accum_out=sD[:, t:t + 1],
                    )

                for t in range(T):
                    r0 = t * P
                    xp = big.tile([P, wp], fp8, tag="xp")
                    nc.sync.dma_start(
                        out=xp[:], in_=costh8[r0:r0 + P, wa + wd:C]
                    )
                    xa = big.tile([P, wa], fp8, tag="xa")
                    nc.sync.dma_start(
                        out=xa[:], in_=costh8[r0:r0 + P, 0:wa]
                    )
                    xd = big.tile([P, wd], fp8, tag="xd")
                    nc.sync.dma_start(
                        out=xd[:], in_=costh8[r0:r0 + P, wa:wa + wd]
                    )
                    codes = cpool.tile([P, wdp], i16, tag="codes")
                    codes_by_t[t] = codes
                    nc.gpsimd.tensor_scalar(
                        out=codes[:, wd:wdp], in0=xp[:],
                        scalar1=SCHRA_A, scalar2=SCHRA_B,
                        op0=Alu.mult, op1=Alu.add,
                    )
                    nc.scalar.activation(
                        out=exp_scr[:], in_=xa[:], func=Act.Exp,
                        bias=neg_sm[:], scale=S,
                        accum_out=sA[:, t:t + 1],
                    )
                    nc.vector.tensor_scalar(
                        out=codes[:, 0:wd], in0=xd[:],
                        scalar1=SCHRA_A, scalar2=SCHRA_B,
                        op0=Alu.mult, op1=Alu.add,
                    )
                    if t > 0:
                        pass2(t - 1)
                pass2(T - 1)

            if loop_reps > 1:
                with tc.For_i(0, loop_reps, 1):
                    for _rep in range(repeat):
                        one_pass()
            for _rep in range(repeat):
                one_pass()

            # --- tail ---
            # target-term math first (needs only cos_t, which the gather
            # produced while the stream ran; keeping these small ACTIVATEs
            # out of the prologue lets ACT start streaming immediately
            # instead of stalling on the gather chain):
            #   delta_e = exp(1 - cos_t);  ct_adj = cos_t - DCOEF * delta_e
            #   e12 = exp(S*(cos_t - M)) - exp(S*(ct_adj - M))
            delta_e = small.tile([P, T], f32)
            nc.scalar.activation(
                out=delta_e[:], in_=cos_t[:], func=Act.Exp, bias=1.0, scale=-1.0
            )
            ct_adj = small.tile([P, T], f32)
            nc.vector.scalar_tensor_tensor(
                out=ct_adj[:], in0=delta_e[:], scalar=-DCOEF, in1=cos_t[:],
                op0=Alu.mult, op1=Alu.add,
            )
            e1 = small.tile([P, T], f32)
            nc.scalar.activation(
                out=e1[:], in_=cos_t[:], func=Act.Exp, bias=neg_sm[:], scale=S
            )
            e2 = small.tile([P, T], f32)
            nc.scalar.activation(
                out=e2[:], in_=ct_adj[:], func=Act.Exp, bias=neg_sm[:], scale=S
            )
            e12 = small.tile([P, T], f32)
            nc.vector.tensor_sub(out=e12[:], in0=e1[:], in1=e2[:])

            # z = DEBIAS_ACT*sA + DEBIAS_DVE*sD - e12.  The final
            # ln(z) - S*ct_adj is finished on the HOST (O(B) work, where the
            # mean-reduction already happens): keeping Ln off the device
            # avoids a second ACT table load (~2.7 us) in the tail, since
            # the stream's Exp set does not contain Ln.
            zd = small.tile([P, T], f32)
            nc.vector.scalar_tensor_tensor(
                out=zd[:], in0=sD[:], scalar=DEBIAS_DVE, in1=e12[:],
                op0=Alu.mult, op1=Alu.subtract,
            )
            z = small.tile([P, T], f32)
            nc.vector.scalar_tensor_tensor(
                out=z[:], in0=sA[:], scalar=DEBIAS_ACT, in1=zd[:],
                op0=Alu.mult, op1=Alu.add,
            )
            nc.sync.dma_start(out=out[:, 0:T], in_=z[:])
            nc.sync.dma_start(out=out[:, T:2 * T], in_=ct_adj[:])

    nc.compile()
    return nc


def _get_nc():
    if "nc" not in _NC_CACHE:
        _NC_CACHE["nc"] = _build_nc()
    return _NC_CACHE["nc"]


def _full_inputs(costh, label):
    """Full (unsharded) input arrays keyed by dram tensor name."""
    costh = np.ascontiguousarray(costh, dtype=np.float32)
    return {
        "costh": costh,
        "costh8": costh.astype(FP8),
        "label": np.ascontiguousarray(label).astype(np.int32),
    }


def _run(costh_np, label_np, trace=False, **spmd_kwargs):
    from concourse.bass_utils import run_bass_kernel_spmd

    nc = _get_nc()
    full = _full_inputs(costh_np, label_np)
    in_maps = [
        {name: arr[k * R:(k + 1) * R] for name, arr in full.items()}
        for k in range(NCORES)
    ]
    # The first execution of a fresh NEFF through the axon tunnel
    # occasionally faults with NRT_EXEC_UNIT_UNRECOVERABLE; failures are
    # loud (exception, never silent corruption), so a bounded retry is safe.
    # A non-finite total also triggers a retry as extra insurance.
    last_exc = None
    for _attempt in range(3):
        try:
            res = run_bass_kernel_spmd(
                nc, in_maps, core_ids=list(range(NCORES)), trace=trace,
                **spmd_kwargs
            )
            total = 0.0
            for r in res.results:
                o = r["out"].astype(np.float64)
                zv, ctv = o[:, :T], o[:, T:]
                total += (np.log(zv) - S * ctv).sum()
            if np.isfinite(total):
                break
            last_exc = RuntimeError("non-finite loss from device")
        except Exception as exc:  # noqa: BLE001
            last_exc = exc
    else:
        raise last_exc
    loss = np.float32(total / B + S * MAXC)
    return loss, res


def kernel(costh, label):
    loss, _ = _run(costh, label)
    return loss


# revision 26
# speedup vs baseline: 1.6200x; 1.1619x over previous
"""DAM-Softmax loss kernel for Trainium2 (Bass/Tile), 8-core data parallel.

Math (per sample b, target t = label[b]):
    cos_t  = costh[b, t]
    delta  = (MARGIN/LAMDA) * exp(1 - cos_t)
    logits = S * costh, with logits[b, t] = S * (cos_t - delta)
    loss_b = logsumexp(logits[b, :]) - S * (cos_t - delta)
    loss   = mean_b loss_b

costh is bounded in [0, 1), so M = 1.0 is the stability shift:
    ssum   = sum_j exp(S*(costh[b,j] - M))
    Z      = ssum - exp(S*(cos_t - M)) + exp(S*(cos_t - delta - M))
    loss_b = S*M + ln(Z) - S*(cos_t - delta)

Performance structure (per core: [1024, 10000] shard, 10.24M exps):

* The bulk ssum term only needs ~1% accuracy (harness tolerance is 2e-2
  on the final scalar; per-element errors average out over 10000
  columns), so the stream tensor is staged in HBM as fp8 e4m3 -- 4x
  fewer bytes than f32.  The fp8 DMA stream (10.24 MB/core) takes
  ~20-25 us/pass on HW -- the roofline the kernel targets.
* The 10.24M elementwise exps are split across FOUR engines (v3,
  _build_v3 -- the graded path):
    - ACT (ScalarE): true exp via activation table with fused row
      accum (~0.78 ns/free-col) on the row-major columns [0, WA3).
    - DVE (VectorE) + Pool (GPSIMD): Schraudolph exp2 pass1
          i16 = rint(x * (S*log2e*128) + (127 - S*log2e)*128)
      (one tensor_scalar each, f32 internal, int16 round-on-write;
      ~1.04 / ~0.65 ns/free-col; bit-exact across both engines,
      verified on HW) over the remaining classes, which are staged
      HOST-TRANSPOSED so samples lie along the free axis.
    - PE (TensorE): the per-sample reduction of the Schraudolph codes.
      bitcast i16 -> bf16 gives 2^z with mantissa-linear interpolation;
      in transposed layout the per-sample sum is a PARTITION-axis
      reduction, i.e. a matmul: ones[128,1]^T @ codes[128,512]
      accumulated over all class-chunks into 2 PSUM banks.  This
      replaced a DVE 4x bitcast-accumulate (pass2) in row-major layout
      that measured ~20 us/pass of DVE serialization; PE does the same
      sums on its own SBUF/PSUM ports in ~3 us of critical path.
  The deterministic quantization/interpolation biases of the two paths
  (true-exp-of-fp8 vs Schraudolph-of-fp8) are removed by constant
  factors computed offline from the rounding rules (equidistribution
  within bins -- input-independent).
* Per-sample target terms (cos_t gather, margin, e1/e2, -S*ct_adj) are
  computed from the untouched f32 input: exact where it matters.
* Final combine z = DEBIAS_ACT*sA + DEBIAS_DVE*sd - e12 and
  ln(z) - S*ct_adj happen on the HOST (O(B) work, where the mean
  already happens): the PE sums live in [1, R] sample-major layout
  while sA/e12 are [128, T] partition-major, and reconciling them
  on-device would cost more than the host loop.

Geometry WA3/GSZ/NDVE = 3856/6/3 balances ACT (~24 us), DVE 3 groups
(~19 us), Pool 5 groups (~20 us), PE (~12 us busy) against the
~20-25 us fp8 DMA stream; measured ~33 us/pass vs ~50-60 us for the
previous ACT+DVE row-major kernel under identical conditions.
(_build_nc, the earlier row-major 3-engine variant, is kept for A/B
timing comparisons via tune.py and is not used by kernel().)

Sharding: batch dim split evenly across 8 NeuronCores (data parallel);
host mean-reduces the per-sample losses.
"""

import numpy as np
import ml_dtypes

NCORES = 8
B, C = 8192, 10000
R = B // NCORES          # rows per core
P = 128                  # SBUF partitions
T = R // P               # row tiles per core
S = 15.0
MARGIN = 0.3
LAMDA = 2.0
DCOEF = MARGIN / LAMDA
MAXC = 1.0               # upper bound of costh (uniform [0,1)) used as exp shift

FP8 = ml_dtypes.float8_e4m3
LOG2E = float(np.log2(np.e))
SCHRA_A = S * LOG2E * 128.0             # pass1: code = rint(x*A + B)
SCHRA_B = (127.0 - S * LOG2E) * 128.0
WA = 4580                # columns handled by ACT (true exp)
WD = 1420                # columns handled by DVE (Schraudolph pass1)
WP = C - WA - WD         # columns handled by Pool (Schraudolph pass1)


def _schraudolph_np(x64):
    """Bit-exact numpy model of the DVE/Pool pass1 + pass2 pipeline."""
    codes = np.rint(x64 * SCHRA_A + SCHRA_B).astype(np.int16)
    return codes.view(ml_dtypes.bfloat16).astype(np.float64)


def _debias():
    """Constant bias factors of the two approximate paths vs true exp,
    for equidistributed in-bin rounding errors (input-independent).

    ACT path: fp8 quantization only.  DVE/Pool path: fp8 quantization +
    Schraudolph mantissa-linear interpolation + int16 rounding.
    """
    x = (np.arange(1 << 20, dtype=np.float64) + 0.5) / (1 << 20)
    xq = x.astype(np.float32).astype(FP8).astype(np.float64)
    num = np.exp(S * (x - 1.0)).sum()
    act = num / np.exp(S * (xq - 1.0)).sum()
    dve = num / _schraudolph_np(xq).sum()
    return float(act), float(dve)


DEBIAS_ACT, DEBIAS_DVE = _debias()

# --- v3 geometry: ACT keeps row-major [0, WA3); the remaining C-WA3
# classes are staged host-transposed in NG grouped tiles of GSZ chunks ---
WA3 = 3856               # ACT columns (chosen so C-WA3 = 6144 = 48*128)
NCHUNK = (C - WA3) // P  # 48 transposed class-chunks of 128
GSZ = 6                  # chunks per group (one DMA / one pass1 per group)
NG = NCHUNK // GSZ       # 8 groups
NDVE = 3                 # groups pass1'd by DVE (rest by Pool)

_NC_CACHE = {}


def _build_nc(repeat=1, big_bufs=4, loop_reps=1, wa=WA, wd=WD, act16=1,
              skip_act=0, skip_dve=0, skip_pool=0, skip_pass2=0, skip_dma=0):
    # skip_* build perf-probe variants that omit one component (numerically
    # wrong outputs; used only by tune.py ablations, never by kernel()).
    # repeat > 1 re-streams the shard `repeat` times inside one NEFF; used by
    # the timing harness to infer per-pass device time from the wall-clock
    # slope (axon dispatch overhead cancels in the difference).  loop_reps > 1
    # additionally wraps the passes in a hardware For_i loop (amplifies
    # device time without growing the NEFF, at ~10% loop-sync tax).
    import concourse.bacc as bacc
    import concourse.bass as bass
    import concourse.mybir as mybir
    import concourse.tile as tile

    f32 = mybir.dt.float32
    fp8 = mybir.dt.float8e4
    bf16 = mybir.dt.bfloat16
    i16 = mybir.dt.int16
    i32 = mybir.dt.int32
    Act = mybir.ActivationFunctionType
    Alu = mybir.AluOpType

    wp = C - wa - wd
    wdp = wd + wp
    nc = bacc.Bacc(None, target_bir_lowering=False, debug=False)

    costh = nc.dram_tensor("costh", [R, C], f32, kind="ExternalInput")
    costh8 = nc.dram_tensor("costh8", [R, C], fp8, kind="ExternalInput")
    label = nc.dram_tensor("label", [R], i32, kind="ExternalInput")
    out = nc.dram_tensor("out", [P, 2 * T], f32, kind="ExternalOutput")

    with tile.TileContext(nc) as tc:
        with (
            tc.tile_pool(name="big", bufs=big_bufs) as big,
            tc.tile_pool(name="codes", bufs=3) as cpool,
            tc.tile_pool(name="small", bufs=1) as small,
        ):
            # bias vector for exp(S*x - S*M) activations
            neg_sm = small.tile([P, 1], f32)
            nc.vector.memset(neg_sm[:], -S * MAXC)

            # --- prologue: gather target cosines cos_t[p, t] = costh[t*P+p, label] ---
            label_sb = small.tile([P, T], i32)
            nc.gpsimd.dma_start(
                out=label_sb[:], in_=label[:].rearrange("(t p) -> p t", p=P)
            )
            # idx[p, t] = (t*P + p) * C + label  (flat element index), computed
            # in f32 (exact: values < 2^24) since iota steps are limited to i16.
            row_i = small.tile([P, T], i32)
            nc.gpsimd.iota(row_i[:], pattern=[[P, T]], base=0, channel_multiplier=1)
            row_f = small.tile([P, T], f32)
            nc.vector.tensor_copy(out=row_f[:], in_=row_i[:])
            lab_f = small.tile([P, T], f32)
            nc.vector.tensor_copy(out=lab_f[:], in_=label_sb[:])
            idx_f = small.tile([P, T], f32)
            nc.vector.scalar_tensor_tensor(
                out=idx_f[:], in0=row_f[:], scalar=float(C), in1=lab_f[:],
                op0=Alu.mult, op1=Alu.add,
            )
            idx = small.tile([P, T], i32)
            nc.vector.tensor_copy(out=idx[:], in_=idx_f[:])
            # one indirect DMA per column: HW honors only one index per
            # partition per gather (multi-column offset APs misbehave on HW)
            cos_t = small.tile([P, T], f32)
            for t in range(T):
                nc.gpsimd.indirect_dma_start(
                    out=cos_t[:, t:t + 1],
                    out_offset=None,
                    in_=costh[:, :],
                    in_offset=bass.IndirectOffsetOnAxis(ap=idx[:, t:t + 1], axis=1),
                )

            # --- main loop: stream fp8 shard; per tile, one DMA of all C
            # columns, then ACT does true exp+accum on cols [0, wa), DVE
            # does Schraudolph pass1 on [wa, wa+wd), Pool on [wa+wd, C);
            # DVE pass2 accumulates both code ranges at 4x ---
            exp_scr = small.tile([P, wa], bf16 if act16 else f32)
            # ACT main out (scratch; only accum_out is consumed; bf16
            # halves ACT's SBUF writes)
            scr16 = small.tile([P, wdp], bf16)     # DVE pass2 main out (scratch)
            sA = small.tile([P, T], f32)
            sD = small.tile([P, T], f32)

            def one_pass():
                # Software-pipelined: pass2 for tile t runs one tile behind
                # the pass1s, so the DVE queue never head-of-line blocks on
                # Pool finishing the current tile's codes.
                codes_by_t = {}

                def pass2(t):
                    nc.vector.tensor_scalar(
                        out=scr16[:], in0=codes_by_t.pop(t)[:].bitcast(bf16),
                        scalar1=1.0, scalar2=0.0, op0=Alu.mult, op1=Alu.add,
                        accum_out=sD[:, t:t + 1],
                    )

                for t in range(T):
                    r0 = t * P
                    if skip_dma and t > 0:
                        codes = cpool.tile([P, wdp], i16, tag="codes")
                        codes_by_t[t] = codes
                        xp, xa, xd = xp0, xa0, xd0
                    else:
                        xp = big.tile([P, wp], fp8, tag="xp")
                        nc.sync.dma_start(
                            out=xp[:], in_=costh8[r0:r0 + P, wa + wd:C]
                        )
                        xa = big.tile([P, wa], fp8, tag="xa")
                        nc.sync.dma_start(
                            out=xa[:], in_=costh8[r0:r0 + P, 0:wa]
                        )
                        xd = big.tile([P, wd], fp8, tag="xd")
                        nc.sync.dma_start(
                            out=xd[:], in_=costh8[r0:r0 + P, wa:wa + wd]
                        )
                        codes = cpool.tile([P, wdp], i16, tag="codes")
                        codes_by_t[t] = codes
                        if skip_dma:
                            xp0, xa0, xd0 = xp, xa, xd
                    if not skip_pool:
                        nc.gpsimd.tensor_scalar(
                            out=codes[:, wd:wdp], in0=xp[:],
                            scalar1=SCHRA_A, scalar2=SCHRA_B,
                            op0=Alu.mult, op1=Alu.add,
                        )
                    if not skip_act:
                        nc.scalar.activation(
                            out=exp_scr[:], in_=xa[:], func=Act.Exp,
                            bias=neg_sm[:], scale=S,
                            accum_out=sA[:, t:t + 1],
                        )
                    if not skip_dve:
                        nc.vector.tensor_scalar(
                            out=codes[:, 0:wd], in0=xd[:],
                            scalar1=SCHRA_A, scalar2=SCHRA_B,
                            op0=Alu.mult, op1=Alu.add,
                        )
                    if not skip_pass2:
                        if t > 0:
                            pass2(t - 1)
                if skip_pass2:
                    codes_by_t.clear()
                    nc.vector.memset(sD[:], 1.0)
                else:
                    pass2(T - 1)
                if skip_act:
                    nc.vector.memset(sA[:], 1.0)

            if loop_reps > 1:
                with tc.For_i(0, loop_reps, 1):
                    for _rep in range(repeat):
                        one_pass()
            for _rep in range(repeat):
                one_pass()

            # --- tail ---
            # target-term math first (needs only cos_t, which the gather
            # produced while the stream ran; keeping these small ACTIVATEs
            # out of the prologue lets ACT start streaming immediately
            # instead of stalling on the gather chain):
            #   delta_e = exp(1 - cos_t);  ct_adj = cos_t - DCOEF * delta_e
            #   e12 = exp(S*(cos_t - M)) - exp(S*(ct_adj - M))
            delta_e = small.tile([P, T], f32)
            nc.scalar.activation(
                out=delta_e[:], in_=cos_t[:], func=Act.Exp, bias=1.0, scale=-1.0
            )
            ct_adj = small.tile([P, T], f32)
            nc.vector.scalar_tensor_tensor(
                out=ct_adj[:], in0=delta_e[:], scalar=-DCOEF, in1=cos_t[:],
                op0=Alu.mult, op1=Alu.add,
            )
            e1 = small.tile([P, T], f32)
            nc.scalar.activation(
                out=e1[:], in_=cos_t[:], func=Act.Exp, bias=neg_sm[:], scale=S
            )
            e2 = small.tile([P, T], f32)
            nc.scalar.activation(
                out=e2[:], in_=ct_adj[:], func=Act.Exp, bias=neg_sm[:], scale=S
            )
            e12 = small.tile([P, T], f32)
            nc.vector.tensor_sub(out=e12[:], in0=e1[:], in1=e2[:])

            # z = DEBIAS_ACT*sA + DEBIAS_DVE*sD - e12.  The final
            # ln(z) - S*ct_adj is finished on the HOST (O(B) work, where the
            # mean-reduction already happens): keeping Ln off the device
            # avoids a second ACT table load (~2.7 us) in the tail, since
            # the stream's Exp set does not contain Ln.
            zd = small.tile([P, T], f32)
            nc.vector.scalar_tensor_tensor(
                out=zd[:], in0=sD[:], scalar=DEBIAS_DVE, in1=e12[:],
                op0=Alu.mult, op1=Alu.subtract,
            )
            z = small.tile([P, T], f32)
            nc.vector.scalar_tensor_tensor(
                out=z[:], in0=sA[:], scalar=DEBIAS_ACT, in1=zd[:],
                op0=Alu.mult, op1=Alu.add,
            )
            nc.sync.dma_start(out=out[:, 0:T], in_=z[:])
            nc.sync.dma_start(out=out[:, T:2 * T], in_=ct_adj[:])

    nc.compile()
    return nc


def _build_v3(repeat=1, loop_reps=1, wa=WA3, gsz=GSZ, ndve=NDVE, big_bufs=3,
              code_bufs=3, skip_pe=0, skip_act=0, skip_sch=0):
    """v3: ACT row-major + host-transposed Schraudolph with PE summation.

    The DVE 4x bitcast-accumulate (pass2) measured ~20 us/pass of DVE
    serialization in the row-major design; staging the Schraudolph
    classes transposed turns the per-sample reduction into a
    partition-axis sum, which the otherwise-idle PE does via
    ones[128,1]^T @ codes[128,512] matmuls accumulated in 2 PSUM banks.
    DVE and Pool then only run pass1 at full rate.
    """
    import concourse.bacc as bacc
    import concourse.bass as bass
    import concourse.mybir as mybir
    import concourse.tile as tile

    f32 = mybir.dt.float32
    fp8 = mybir.dt.float8e4
    bf16 = mybir.dt.bfloat16
    i16 = mybir.dt.int16
    i32 = mybir.dt.int32
    Act = mybir.ActivationFunctionType
    Alu = mybir.AluOpType

    nchunk = (C - wa) // P
    assert (C - wa) % P == 0
    ng = nchunk // gsz
    assert nchunk % gsz == 0
    gw = gsz * 1024          # free width of a group tile
    nmm = 2 * gsz            # matmuls per group ([128, 512] each)

    nc = bacc.Bacc(None, target_bir_lowering=False, debug=False)

    costh = nc.dram_tensor("costh", [R, C], f32, kind="ExternalInput")
    costh8 = nc.dram_tensor("costh8", [R, C], fp8, kind="ExternalInput")
    costh8t = nc.dram_tensor("costh8t", [ng, P, gw], fp8, kind="ExternalInput")
    label = nc.dram_tensor("label", [R], i32, kind="ExternalInput")
    out = nc.dram_tensor("out", [P, 3 * T], f32, kind="ExternalOutput")
    out2 = nc.dram_tensor("out2", [1, R], f32, kind="ExternalOutput")

    with tile.TileContext(nc) as tc:
        with (
            tc.tile_pool(name="big", bufs=big_bufs) as big,
            tc.tile_pool(name="codes", bufs=code_bufs) as cpool,
            tc.tile_pool(name="small", bufs=1) as small,
            tc.tile_pool(name="psum", bufs=1, space=bass.MemorySpace.PSUM) as pp,
        ):
            neg_sm = small.tile([P, 1], f32)
            nc.vector.memset(neg_sm[:], -S * MAXC)
            ones = small.tile([P, 1], bf16)
            nc.vector.memset(ones[:], 1.0)
            ps0 = pp.tile([1, 512], f32)
            ps1 = pp.tile([1, 512], f32)
            ps = [ps0, ps1]

            # --- prologue: gather target cosines (unchanged from v2) ---
            label_sb = small.tile([P, T], i32)
            nc.gpsimd.dma_start(
                out=label_sb[:], in_=label[:].rearrange("(t p) -> p t", p=P)
            )
            row_i = small.tile([P, T], i32)
            nc.gpsimd.iota(row_i[:], pattern=[[P, T]], base=0, channel_multiplier=1)
            row_f = small.tile([P, T], f32)
            nc.vector.tensor_copy(out=row_f[:], in_=row_i[:])
            lab_f = small.tile([P, T], f32)
            nc.vector.tensor_copy(out=lab_f[:], in_=label_sb[:])
            idx_f = small.tile([P, T], f32)
            nc.vector.scalar_tensor_tensor(
                out=idx_f[:], in0=row_f[:], scalar=float(C), in1=lab_f[:],
                op0=Alu.mult, op1=Alu.add,
            )
            idx = small.tile([P, T], i32)
            nc.vector.tensor_copy(out=idx[:], in_=idx_f[:])
            cos_t = small.tile([P, T], f32)
            for t in range(T):
                nc.gpsimd.indirect_dma_start(
                    out=cos_t[:, t:t + 1],
                    out_offset=None,
                    in_=costh[:, :],
                    in_offset=bass.IndirectOffsetOnAxis(ap=idx[:, t:t + 1], axis=1),
                )

            exp_scr = small.tile([P, wa], bf16)   # ACT main out (scratch)
            sA = small.tile([P, T], f32)

            # group g -> pass1 engine: interleave DVE among Pool groups
            dve_groups = set()
            if ndve:
                stride = max(1, ng // ndve)
                g = 1
                while len(dve_groups) < ndve:
                    dve_groups.add(g % ng)
                    g += stride

            def one_pass():
                for g in range(ng):
                    if not skip_sch:
                        xg = big.tile([P, gw], fp8, tag="xg")
                        nc.sync.dma_start(out=xg[:], in_=costh8t[g])
                        codes = cpool.tile([P, gw], i16, tag="codes")
                        eng = nc.vector if g in dve_groups else nc.gpsimd
                        eng.tensor_scalar(
                            out=codes[:], in0=xg[:],
                            scalar1=SCHRA_A, scalar2=SCHRA_B,
                            op0=Alu.mult, op1=Alu.add,
                        )
                    if g < T and not skip_act:
                        t = g
                        xa = big.tile([P, wa], fp8, tag="xa")
                        nc.sync.dma_start(
                            out=xa[:], in_=costh8[t * P:(t + 1) * P, 0:wa]
                        )
                        nc.scalar.activation(
                            out=exp_scr[:], in_=xa[:], func=Act.Exp,
                            bias=neg_sm[:], scale=S,
                            accum_out=sA[:, t:t + 1],
                        )
                    if not (skip_pe or skip_sch):
                        for j in range(nmm):
                            nc.tensor.matmul(
                                ps[j % 2][:],
                                ones[:],
                                codes[:, j * 512:(j + 1) * 512].bitcast(bf16),
                                start=(g == 0 and j < 2),
                                stop=(g == ng - 1 and j >= nmm - 2),
                            )

            if loop_reps > 1:
                with tc.For_i(0, loop_reps, 1):
                    for _rep in range(repeat):
                        one_pass()
            for _rep in range(repeat):
                one_pass()

            # --- tail: per-sample target terms + PSUM evacuation ---
            delta_e = small.tile([P, T], f32)
            nc.scalar.activation(
                out=delta_e[:], in_=cos_t[:], func=Act.Exp, bias=1.0, scale=-1.0
            )
            ct_adj = small.tile([P, T], f32)
            nc.vector.scalar_tensor_tensor(
                out=ct_adj[:], in0=delta_e[:], scalar=-DCOEF, in1=cos_t[:],
                op0=Alu.mult, op1=Alu.add,
            )
            e1 = small.tile([P, T], f32)
            nc.scalar.activation(
                out=e1[:], in_=cos_t[:], func=Act.Exp, bias=neg_sm[:], scale=S
            )
            e2 = small.tile([P, T], f32)
            nc.scalar.activation(
                out=e2[:], in_=ct_adj[:], func=Act.Exp, bias=neg_sm[:], scale=S
            )
            e12 = small.tile([P, T], f32)
            nc.vector.tensor_sub(out=e12[:], in0=e1[:], in1=e2[:])

            nc.sync.dma_start(out=out[:, 0:T], in_=sA[:])
            nc.sync.dma_start(out=out[:, T:2 * T], in_=ct_adj[:])
            nc.sync.dma_start(out=out[:, 2 * T:3 * T], in_=e12[:])
            if skip_act:
                nc.vector.memset(sA[:], 1.0)
            ev = small.tile([1, R], f32)
            if skip_pe or skip_sch:
                nc.vector.memset(ev[:], 1.0)
            else:
                for h in range(2):
                    nc.vector.tensor_copy(
                        out=ev[:, h * 512:(h + 1) * 512], in_=ps[h][:]
                    )
            nc.sync.dma_start(out=out2[:], in_=ev[:])

    nc.compile()
    return nc


def _get_nc():
    if "nc" not in _NC_CACHE:
        _NC_CACHE["nc"] = _build_v3()
    return _NC_CACHE["nc"]


def _full_inputs(costh, label, wa=WA3, gsz=GSZ):
    """Full (unsharded) input arrays keyed by dram tensor name."""
    nchunk = (C - wa) // P
    ng = nchunk // gsz
    costh = np.ascontiguousarray(costh, dtype=np.float32)
    costh8 = costh.astype(FP8)
    # v3 transposed staging: per core, classes [wa, C) of its row shard,
    # grouped so each [P, gsz*R] group tile is one contiguous-line DMA:
    # costh8t[k*ng + g, p, r*R + f] = costh8[k*R + f, wa + (g*gsz + r)*P + p]
    xt = costh8[:, wa:].reshape(NCORES, R, nchunk, P)
    costh8t = np.ascontiguousarray(
        xt.transpose(0, 2, 3, 1)                   # [k, chunk, p, f]
          .reshape(NCORES, ng, gsz, P, R)
          .transpose(0, 1, 3, 2, 4)                # [k, g, p, r, f]
          .reshape(NCORES * ng, P, gsz * R)
    )
    return {
        "costh": costh,
        "costh8": costh8,
        "costh8t": costh8t,
        "label": np.ascontiguousarray(label).astype(np.int32),
    }


def _per_core_maps(full):
    """Slice full input arrays into per-core maps (axis-0 shards)."""
    rows = {"costh": R, "costh8": R, "label": R,
            "costh8t": full["costh8t"].shape[0] // NCORES}
    return [
        {name: arr[k * rows[name]:(k + 1) * rows[name]]
         for name, arr in full.items()}
        for k in range(NCORES)
    ]


def _run(costh_np, label_np, trace=False, **spmd_kwargs):
    from concourse.bass_utils import run_bass_kernel_spmd

    nc = _get_nc()
    full = _full_inputs(costh_np, label_np)
    in_maps = _per_core_maps(full)
    # The first execution of a fresh NEFF through the axon tunnel
    # occasionally faults with NRT_EXEC_UNIT_UNRECOVERABLE; failures are
    # loud (exception, never silent corruption), so a bounded retry is safe.
    # A non-finite total also triggers a retry as extra insurance.
    last_exc = None
    for _attempt in range(3):
        try:
            res = run_bass_kernel_spmd(
                nc, in_maps, core_ids=list(range(NCORES)), trace=trace,
                **spmd_kwargs
            )
            total = 0.0
            for r in res.results:
                o = r["out"].astype(np.float64)
                sa, ctv, e12 = o[:, :T], o[:, T:2 * T], o[:, 2 * T:3 * T]
                sd = r["out2"].astype(np.float64).reshape(T, P).T
                zv = DEBIAS_ACT * sa + DEBIAS_DVE * sd - e12
                total += (np.log(zv) - S * ctv).sum()
            if np.isfinite(total):
                break
            last_exc = RuntimeError("non-finite loss from device")
        except Exception as exc:  # noqa: BLE001
            last_exc = exc
    else:
        raise last_exc
    loss = np.float32(total / B + S * MAXC)
    return loss, res


def kernel(costh, label):
    loss, _ = _run(costh, label)
    return loss
